# revision 1
# baseline (speedup 1.0000x reference)
"""Trainium2 Bass kernel for nn_Encoder (B=4, S=2048, D=512, H=8 self-attention).

Sharding over 8 NeuronCores: core c -> (batch b = c//2, head-group hg = c%2).
Each core computes, for its batch and its 4 heads, the full attention block
plus a partial output projection y_part = attn_out @ Wo[group rows]. The host
sums the two partial y tensors per batch (the head-concat + Wo projection is
linear in the head groups).

Device-side layout (everything transposed so the contraction dim is always on
SBUF partitions):
  xT [D, S]         : host-pretransposed input, d on partitions (4 chunks)
  KT/QT per pair    : [128, S] = [2 heads' e, s/t], from W.T @ x matmuls
  scoresT [s, t]    : s on partitions -> key-padding mask becomes a
                      per-partition bias AP fused into the ACT Exp instruction
                      (scale=1/sqrt(DH) fused there too)
  V' [s, e + ones]  : appended ones column makes the softmax denominator fall
                      out of the attnV matmul (psum row 64) for free
  outT [he, t]      : exactly the lhsT layout the Wo projection wants

Matmul inputs are float32r (fp32 data, full PE rate at free-dim >= 256).
"""

import ml_dtypes
import numpy as np

import concourse.mybir as mybir
import concourse.tile as tile
from concourse import bacc
from concourse.bass_utils import run_bass_kernel_spmd

B, S, D, H = 4, 2048, 512, 8
DH = D // H          # 64
HPC = H // 2         # 4 heads per core
HE = HPC * DH        # 256 output-proj rows per core
T = S                # full query length per core
NDC = D // 128       # 4 contraction chunks for projections
NST = S // 128       # 16 key tiles
MASK_NUM = 1.0e9
N_CORES = 8

f32 = mybir.dt.float32
f32r = mybir.dt.float32r
bf16 = mybir.dt.bfloat16
EXP = mybir.ActivationFunctionType.Exp


def build_nc():
    nc = bacc.Bacc("TRN2", target_bir_lowering=False, debug=False, num_devices=1)

    xT = nc.dram_tensor("xT", [D, S], bf16, kind="ExternalInput").ap()
    wq = nc.dram_tensor("wq", [D, HE], bf16, kind="ExternalInput").ap()
    wk = nc.dram_tensor("wk", [D, HE], bf16, kind="ExternalInput").ap()
    wv = nc.dram_tensor("wv", [D, HE], bf16, kind="ExternalInput").ap()
    wo = nc.dram_tensor("wo", [HE, D], bf16, kind="ExternalInput").ap()
    mb = nc.dram_tensor("mbias", [S], f32, kind="ExternalInput").ap()
    y = nc.dram_tensor("y", [T, D], f32, kind="ExternalOutput").ap()

    with tile.TileContext(nc) as tc:
        with (
            tc.tile_pool(name="const", bufs=1) as const,
            tc.tile_pool(name="psA", bufs=4, space="PSUM") as psA,
            tc.tile_pool(name="psS", bufs=2, space="PSUM") as psS,
            tc.tile_pool(name="attnT", bufs=12) as at_pool,
            tc.tile_pool(name="yout", bufs=3) as y_pool,
            tc.tile_pool(name="recip", bufs=4) as r_pool,
            tc.tile_pool(name="recipb", bufs=4) as rb_pool,
            tc.tile_pool(name="avsb", bufs=8) as av_pool,
            tc.tile_pool(name="sums", bufs=2) as sums_pool,
        ):
            # ---- Stage A: loads -------------------------------------------
            xT_sb = const.tile([128, NDC, S], bf16, tag="xT")
            wq_sb = const.tile([128, NDC, HE], bf16, tag="wq")
            wk_sb = const.tile([128, NDC, HE], bf16, tag="wk")
            wv_sb = const.tile([128, NDC, HE], bf16, tag="wv")
            wo_sb = const.tile([128, HE // 128, D], bf16, tag="wo")
            mb_sb = const.tile([128, NST], f32, tag="mb")
            xT_r = xT.rearrange("(c p) s -> c p s", p=128)
            nc.sync.dma_start(xT_sb[:, 0, :], xT_r[0])
            nc.sync.dma_start(wk_sb[:], wk.rearrange("(c p) n -> p c n", p=128))
            nc.sync.dma_start(wq_sb[:], wq.rearrange("(c p) n -> p c n", p=128))
            nc.sync.dma_start(xT_sb[:, 1, :], xT_r[1])
            nc.sync.dma_start(wv_sb[:], wv.rearrange("(c p) n -> p c n", p=128))
            nc.sync.dma_start(xT_sb[:, 2, :], xT_r[2])
            nc.sync.dma_start(wo_sb[:], wo.rearrange("(c p) n -> p c n", p=128))
            nc.sync.dma_start(xT_sb[:, 3, :], xT_r[3])
            nc.sync.dma_start(mb_sb[:], mb.rearrange("(j p) -> p j", p=128))

            # V' tiles: [s-tile][local head][DH + ones column]
            v_sb = const.tile([128, NST, HPC, DH + 1], bf16, tag="v")
            nc.gpsimd.memset(v_sb[:, :, :, DH : DH + 1], 1.0)

            # ---- Stage B: projections -------------------------------------
            # KT/QT per head pair pp: [128 (2 heads' e), S]
            kt_sb = [
                const.tile([128, S], bf16, tag=f"kt{pp}", name=f"kt{pp}")
                for pp in range(2)
            ]
            qt_sb = [
                const.tile([128, S], bf16, tag=f"qt{pp}", name=f"qt{pp}")
                for pp in range(2)
            ]

            def emit_proj(pp):
                for w_sb, dst in ((wk_sb, kt_sb[pp]), (wq_sb, qt_sb[pp])):
                    for sc in range(S // 512):
                        ps = psA.tile([128, 512], f32, tag="mm", name="proj_ps")
                        for dc in range(NDC):
                            nc.tensor.matmul(
                                ps[:],
                                lhsT=w_sb[:, dc, pp * 128 : (pp + 1) * 128],
                                rhs=xT_sb[:, dc, sc * 512 : (sc + 1) * 512],
                                start=(dc == 0),
                                stop=(dc == NDC - 1),
                            )
                        nc.vector.tensor_copy(dst[:, sc * 512 : (sc + 1) * 512], ps[:])

            emit_proj(0)

            def emit_v_group(g):
                for vst in range(g * 4, (g + 1) * 4):
                    ps = psA.tile([128, HE], f32, tag="mm", name="v_ps")
                    for dc in range(NDC):
                        nc.tensor.matmul(
                            ps[:],
                            lhsT=xT_sb[:, dc, vst * 128 : (vst + 1) * 128],
                            rhs=wv_sb[:, dc, :],
                            start=(dc == 0),
                            stop=(dc == NDC - 1),
                        )
                    nc.vector.tensor_copy(
                        v_sb[:, vst, :, 0:DH],
                        ps[:].rearrange("p (h e) -> p h e", e=DH),
                    )

            # ---- Stage C: attention ---------------------------------------
            # outT [he, t] laid out as [128, 2, T]: chunk pp, rows h2*64.
            # Phase = (th, pp). Normalize + Wo of phase P are deferred into
            # phase P+1 so boundaries never stall PE or ACT. pp=1 projections
            # are emitted at the first phase boundary, off the prologue.
            outT_sb = const.tile([128, HE // 128, T], bf16, tag="outT")

            def emit_normalize(pend):
                th_, pp_, av_sbs_ = pend
                sums = sums_pool.tile([97, 512], f32, tag="sums", name="sums")
                nc.gpsimd.memset(sums[:], 1.0)
                for h2 in range(2):
                    for tw in range(2):
                        k = 32 * (h2 * 2 + tw)
                        nc.vector.tensor_copy(
                            sums[k : k + 1, :],
                            av_sbs_[(h2, tw)][DH : DH + 1, :],
                        )
                recips = sums_pool.tile([97, 512], f32, tag="recips", name="recips")
                nc.vector.reciprocal(recips[:], sums[:])
                for h2 in range(2):
                    for tw in range(2):
                        k = 32 * (h2 * 2 + tw)
                        tcol = th_ * 1024 + tw * 512
                        r_t = r_pool.tile([1, 512], f32, tag="r", name="r_t")
                        nc.vector.tensor_copy(r_t[0:1, :], recips[k : k + 1, :])
                        rb_t = rb_pool.tile([64, 512], f32, tag="rb", name="rb_t")
                        nc.gpsimd.partition_broadcast(rb_t[:], r_t[0:1, :])
                        nc.vector.tensor_mul(
                            outT_sb[h2 * 64 : (h2 + 1) * 64, pp_, tcol : tcol + 512],
                            av_sbs_[(h2, tw)][0:DH, :],
                            rb_t[:],
                        )

            def emit_wo(th_):
                for tt in range(th_ * 8, (th_ + 1) * 8):
                    ps = psA.tile([128, 512], f32, tag="mm", name="y_ps")
                    for c in range(HE // 128):
                        nc.tensor.matmul(
                            ps[:],
                            lhsT=outT_sb[:, c, tt * 128 : (tt + 1) * 128],
                            rhs=wo_sb[:, c, :],
                            start=(c == 0),
                            stop=(c == HE // 128 - 1),
                        )
                    y_sb = y_pool.tile([128, 512], f32, tag="y", name="y_sb")
                    nc.vector.tensor_copy(y_sb[:], ps[:])
                    nc.sync.dma_start(y[tt * 128 : (tt + 1) * 128, :], y_sb[:])

            pending = None
            phases = [(th, pp) for th in range(T // 1024) for pp in range(2)]
            for phase_i, (th, pp) in enumerate(phases):
                defer_v = phase_i == 0
                av = None
                if not defer_v:
                    av = [
                        [
                            psA.tile([128, 512], f32, tag="mm", name=f"av{h2}_{tw}")
                            for tw in range(2)
                        ]
                        for h2 in range(2)
                    ]
                deferred_at = []

                def emit_attnv(at_, st_, h2_):
                    h = 2 * pp + h2_
                    for tw in range(2):
                        nc.tensor.matmul(
                            av[h2_][tw][0 : DH + 1, :],
                            lhsT=v_sb[:, st_, h, :],
                            rhs=at_[:, tw * 512 : (tw + 1) * 512],
                            start=(st_ == 0),
                            stop=(st_ == NST - 1),
                        )

                for st in range(NST):
                    sc_ps = [
                        psS.tile([128, 1024], f32, tag="sc", name=f"sc_ps{h2}")
                        for h2 in range(2)
                    ]
                    for tw in range(2):
                        for h2 in range(2):
                            off = h2 * 64
                            tcol = th * 1024 + tw * 512
                            nc.tensor.matmul(
                                sc_ps[h2][:, tw * 512 : (tw + 1) * 512],
                                lhsT=kt_sb[pp][
                                    off : off + 64, st * 128 : (st + 1) * 128
                                ],
                                rhs=qt_sb[pp][off : off + 64, tcol : tcol + 512],
                                start=True,
                                stop=True,
                            )
                    for h2 in range(2):
                        at = at_pool.tile([128, 1024], bf16, tag="at", name="at")
                        nc.scalar.activation(
                            at[:],
                            sc_ps[h2][:],
                            EXP,
                            bias=mb_sb[:, st : st + 1],
                            scale=float(1.0 / np.sqrt(DH)),
                        )
                        if defer_v and st < 4:
                            deferred_at.append((at, st, h2))
                        else:
                            emit_attnv(at, st, h2)
                    if defer_v and st < 4:
                        emit_v_group(st)
                    if defer_v and st == 3:
                        av = [
                            [
                                psA.tile(
                                    [128, 512], f32, tag="mm", name=f"av{h2}_{tw}"
                                )
                                for tw in range(2)
                            ]
                            for h2 in range(2)
                        ]
                        for at_, st_, h2_ in deferred_at:
                            emit_attnv(at_, st_, h2_)
                        deferred_at = []
                    if st == 1 and pending is not None:
                        emit_normalize(pending)
                    if st == 8 and pending is not None:
                        if pending[1] == 1:
                            emit_wo(pending[0])
                        pending = None
                if phase_i < len(phases) - 1:
                    av_sbs = {}
                    for h2 in range(2):
                        for tw in range(2):
                            av_sb = av_pool.tile(
                                [DH + 1, 512], f32, tag="avsb", name=f"av_sb{h2}_{tw}"
                            )
                            nc.vector.tensor_copy(av_sb[:], av[h2][tw][0 : DH + 1, :])
                            av_sbs[(h2, tw)] = av_sb
                    pending = (th, pp, av_sbs)
                else:
                    pending = (th, pp, av)  # last phase: normalize reads PSUM
                if phase_i == 0:
                    emit_proj(1)

            # tail: pipeline normalize and Wo by tw halves; recip first,
            # all reads straight from the attnV psum (no staging copies)
            th_, pp_, av_ = pending
            sums = sums_pool.tile([97, 512], f32, tag="sums", name="sums")
            nc.gpsimd.memset(sums[:], 1.0)
            for h2 in range(2):
                for tw in range(2):
                    k = 32 * (h2 * 2 + tw)
                    nc.vector.tensor_copy(
                        sums[k : k + 1, :], av_[h2][tw][DH : DH + 1, :]
                    )
            recips = sums_pool.tile([97, 512], f32, tag="recips", name="recips")
            nc.vector.reciprocal(recips[:], sums[:])
            for tw in range(2):
                for h2 in range(2):
                    k = 32 * (h2 * 2 + tw)
                    tcol = th_ * 1024 + tw * 512
                    r_t = r_pool.tile([1, 512], f32, tag="r", name="r_t")
                    nc.vector.tensor_copy(r_t[0:1, :], recips[k : k + 1, :])
                    rb_t = rb_pool.tile([64, 512], f32, tag="rb", name="rb_t")
                    nc.gpsimd.partition_broadcast(rb_t[:], r_t[0:1, :])
                    nc.vector.tensor_mul(
                        outT_sb[h2 * 64 : (h2 + 1) * 64, pp_, tcol : tcol + 512],
                        av_[h2][tw][0:DH, :],
                        rb_t[:],
                    )
                for tt in range(th_ * 8 + tw * 4, th_ * 8 + (tw + 1) * 4):
                    ps = psA.tile([128, 512], f32, tag="mm", name="y_ps")
                    for c in range(HE // 128):
                        nc.tensor.matmul(
                            ps[:],
                            lhsT=outT_sb[:, c, tt * 128 : (tt + 1) * 128],
                            rhs=wo_sb[:, c, :],
                            start=(c == 0),
                            stop=(c == HE // 128 - 1),
                        )
                    y_sb = y_pool.tile([128, 512], f32, tag="y", name="y_sb")
                    nc.vector.tensor_copy(y_sb[:], ps[:])
                    nc.sync.dma_start(y[tt * 128 : (tt + 1) * 128, :], y_sb[:])

    nc.compile()
    return nc


_NC_CACHE = None


def _get_nc():
    global _NC_CACHE
    if _NC_CACHE is None:
        _NC_CACHE = build_nc()
    return _NC_CACHE


def make_in_maps(x, mask, Wq, Wk, Wv, Wo):
    bf = ml_dtypes.bfloat16
    xT = np.ascontiguousarray(x.transpose(0, 2, 1)).astype(bf)  # [B, D, S]
    # [H, D, DH] -> [D, H*DH]
    wq_f = np.ascontiguousarray(Wq.transpose(1, 0, 2).reshape(D, H * DH))
    wk_f = np.ascontiguousarray(Wk.transpose(1, 0, 2).reshape(D, H * DH))
    wv_f = np.ascontiguousarray(Wv.transpose(1, 0, 2).reshape(D, H * DH))
    mb = np.where(mask > 0, 0.0, -MASK_NUM).astype(np.float32)  # [B, S]
    in_maps = []
    for c in range(N_CORES):
        b, hg = c // 2, c % 2
        cols = slice(hg * HE, (hg + 1) * HE)
        in_maps.append(
            {
                "xT": xT[b],
                "wq": np.ascontiguousarray(wq_f[:, cols]).astype(bf),
                "wk": np.ascontiguousarray(wk_f[:, cols]).astype(bf),
                "wv": np.ascontiguousarray(wv_f[:, cols]).astype(bf),
                "wo": np.ascontiguousarray(Wo[cols, :]).astype(bf),
                "mbias": mb[b],
            }
        )
    return in_maps


def combine_results(results):
    y = np.zeros((B, S, D), np.float32)
    for c in range(N_CORES):
        y[c // 2] += results[c]["y"]
    return y


def kernel(x, mask, Wq, Wk, Wv, Wo):
    nc = _get_nc()
    in_maps = make_in_maps(
        np.asarray(x, np.float32),
        np.asarray(mask),
        np.asarray(Wq, np.float32),
        np.asarray(Wk, np.float32),
        np.asarray(Wv, np.float32),
        np.asarray(Wo, np.float32),
    )
    res = run_bass_kernel_spmd(nc, in_maps, core_ids=list(range(N_CORES)))
    return combine_results(res.results)



# revision 6
# speedup vs baseline: 1.4488x; 1.4488x over previous
"""Trainium2 Bass kernel for nn_Encoder (B=4, S=2048, D=512, H=8 self-attention).

Sharding over 8 NeuronCores: core c -> (batch b = c//2, head-group hg = c%2).
Each core computes, for its batch and its 4 heads, the full attention block
plus a partial output projection y_part = attn_out @ Wo[group rows]. The host
sums the two partial y tensors per batch.

Key compaction: the key-padding mask kills ~half the keys (their attention
weight is exactly exp(-1e9) = 0). The host permutes each batch's sequence so
valid keys come first; K/V projections, scores, exp and attnV run only over
NKT = ceil(n_valid/128) key tiles instead of 16. Queries stay full (the
output rows are un-permuted on the host).

Device-side layout (contraction dim always on SBUF partitions):
  xT [D, S]         : host-permuted input, d on partitions (4 chunks)
  KT/QT per pair    : [128, SV] / [128, S] = [2 heads' e, s/t]
  scoresT [s, t]    : s on partitions -> key-padding mask becomes a
                      per-partition bias AP fused into the ACT Exp instruction
                      (scale=1/sqrt(DH) fused there too)
  V' [s, e + ones]  : appended ones column makes the softmax denominator fall
                      out of the attnV matmul (psum row 64) for free
  outT [he, t]      : exactly the lhsT layout the Wo projection wants

Schedule: ACT (exp) is the bottleneck engine (1 elem/lane/cycle); the kernel
is organized to keep it 100% busy: scores pairs are emitted adjacently so the
two heads' 64-contraction matmuls run concurrently in different PE row
groups; projection/Wo psum tiles share the av tag and are emitted at phase
boundaries where the slot rotation has free slots; attnV drains through an
`at` backlog so PE fillers never stall the exp pipeline.
"""

import ml_dtypes
import numpy as np

import concourse.mybir as mybir
import concourse.tile as tile
from concourse import bacc
from concourse.bass_utils import run_bass_kernel_spmd

B, S, D, H = 4, 2048, 512, 8
DH = D // H          # 64
HPC = H // 2         # 4 heads per core
HE = HPC * DH        # 256 output-proj rows per core
T = S                # full query length per core
NDC = D // 128       # 4 contraction chunks for projections
MASK_NUM = 1.0e9
N_CORES = 8

f32 = mybir.dt.float32
bf16 = mybir.dt.bfloat16
EXP = mybir.ActivationFunctionType.Exp
SCALE = float(1.0 / np.sqrt(DH))


def build_nc(nkt):
    SV = nkt * 128
    nc = bacc.Bacc("TRN2", target_bir_lowering=False, debug=False, num_devices=1)

    xT = nc.dram_tensor("xT", [D, S], bf16, kind="ExternalInput").ap()
    wq = nc.dram_tensor("wq", [D, HE], bf16, kind="ExternalInput").ap()
    wk = nc.dram_tensor("wk", [D, HE], bf16, kind="ExternalInput").ap()
    wv = nc.dram_tensor("wv", [D, HE], bf16, kind="ExternalInput").ap()
    wo = nc.dram_tensor("wo", [HE, D], bf16, kind="ExternalInput").ap()
    mb = nc.dram_tensor("mbias", [SV], f32, kind="ExternalInput").ap()
    y = nc.dram_tensor("y", [T, D], f32, kind="ExternalOutput").ap()

    # K-projection column slices over the compacted width
    kslices = []
    c0 = 0
    while c0 < SV:
        w = min(512, SV - c0)
        kslices.append((c0, w))
        c0 += w

    with tile.TileContext(nc) as tc:
        with (
            tc.tile_pool(name="const", bufs=1) as const,
            tc.tile_pool(name="psS", bufs=2, space="PSUM") as psS,
            tc.tile_pool(name="psA", bufs=4, space="PSUM") as psA,
            tc.tile_pool(name="attnT", bufs=16) as at_pool,
            tc.tile_pool(name="yout", bufs=4) as y_pool,
            tc.tile_pool(name="recip", bufs=4) as r_pool,
            tc.tile_pool(name="recipb", bufs=4) as rb_pool,
            tc.tile_pool(name="sums", bufs=2) as sums_pool,
        ):
            # ---- HAM warmup: keep PE busy during the DMA prologue so the
            # clock gate is at 8/8 when real matmuls arrive.
            warm_sb = const.tile([128, 512], bf16, tag="warm")
            nc.gpsimd.memset(warm_sb[:], 0.0)
            warm_ps = psA.tile([128, 512], f32, tag="mm", name="warm_ps")
            for _ in range(18):
                nc.tensor.matmul(
                    warm_ps[:], lhsT=warm_sb[:, 0:128], rhs=warm_sb[:],
                    start=True, stop=True,
                )

            # ---- ACT table preload: a dummy exp so the Exp spline tables DMA
            # in during the prologue instead of before the first real exp.
            warm_act = const.tile([1, 2], f32, tag="warm_act")
            nc.gpsimd.memset(warm_act[:], 0.0)
            nc.scalar.activation(warm_act[0:1, 0:1], warm_act[0:1, 1:2], EXP)

            # ---- DMA loads, critical-path order -------------------------
            xT_sb = const.tile([128, NDC, S], bf16, tag="xT")
            wq_sb = const.tile([128, NDC, HE], bf16, tag="wq")
            wk_sb = const.tile([128, NDC, HE], bf16, tag="wk")
            wv_sb = const.tile([128, NDC, HE], bf16, tag="wv")
            wo_sb = const.tile([128, HE // 128, D], bf16, tag="wo")
            mb_sb = const.tile([128, nkt], f32, tag="mb")
            xT_r = xT.rearrange("(c p) s -> c p s", p=128)
            nc.sync.dma_start(wk_sb[:], wk.rearrange("(c p) n -> p c n", p=128))
            for dc in range(NDC):
                nc.sync.dma_start(xT_sb[:, dc, 0:SV], xT_r[dc][:, 0:SV])
            nc.sync.dma_start(wq_sb[:], wq.rearrange("(c p) n -> p c n", p=128))
            nc.sync.dma_start(mb_sb[:], mb.rearrange("(j p) -> p j", p=128))
            nc.sync.dma_start(wv_sb[:], wv.rearrange("(c p) n -> p c n", p=128))
            if SV < S:
                for dc in range(NDC):
                    nc.sync.dma_start(xT_sb[:, dc, SV:S], xT_r[dc][:, SV:S])
            nc.sync.dma_start(wo_sb[:], wo.rearrange("(c p) n -> p c n", p=128))

            # V' tiles: [s-tile][local head][DH + ones column]
            v_sb = const.tile([128, nkt, HPC, DH + 1], bf16, tag="v")
            nc.gpsimd.memset(v_sb[:, :, :, DH : DH + 1], 1.0)

            kt_sb = [
                const.tile([128, SV], bf16, tag=f"kt{pp}", name=f"kt{pp}")
                for pp in range(2)
            ]
            qt_sb = [
                const.tile([128, T], bf16, tag=f"qt{pp}", name=f"qt{pp}")
                for pp in range(2)
            ]
            outT_sb = const.tile([128, HE // 128, T], bf16, tag="outT")

            # ---- projection emitters (psum shares the "mm" tag) ----------
            def emit_kt(pp):
                for c0, w in kslices:
                    ps = psA.tile([128, 512], f32, tag="mm", name="kproj_ps")
                    for dc in range(NDC):
                        nc.tensor.matmul(
                            ps[:, 0:w],
                            lhsT=wk_sb[:, dc, pp * 128 : (pp + 1) * 128],
                            rhs=xT_sb[:, dc, c0 : c0 + w],
                            start=(dc == 0),
                            stop=(dc == NDC - 1),
                        )
                    nc.vector.tensor_copy(kt_sb[pp][:, c0 : c0 + w], ps[:, 0:w])

            def emit_qt(pp, th):
                for sc_i in range(2):
                    t0 = th * 1024 + sc_i * 512
                    ps = psA.tile([128, 512], f32, tag="mm", name="qproj_ps")
                    for dc in range(NDC):
                        nc.tensor.matmul(
                            ps[:],
                            lhsT=wq_sb[:, dc, pp * 128 : (pp + 1) * 128],
                            rhs=xT_sb[:, dc, t0 : t0 + 512],
                            start=(dc == 0),
                            stop=(dc == NDC - 1),
                        )
                    nc.vector.tensor_copy(qt_sb[pp][:, t0 : t0 + 512], ps[:])

            def emit_v(vst):
                ps = psA.tile([128, 512], f32, tag="mm", name="vproj_ps")
                for dc in range(NDC):
                    nc.tensor.matmul(
                        ps[:, 0:HE],
                        lhsT=xT_sb[:, dc, vst * 128 : (vst + 1) * 128],
                        rhs=wv_sb[:, dc, :],
                        start=(dc == 0),
                        stop=(dc == NDC - 1),
                    )
                nc.vector.tensor_copy(
                    v_sb[:, vst, :, 0:DH],
                    ps[:, 0:HE].rearrange("p (h e) -> p h e", e=DH),
                )

            # ---- attention emitters --------------------------------------
            def emit_scores(pp, th, st):
                scps = [
                    psS.tile([128, 1024], f32, tag="sc", name=f"sc{h2}")
                    for h2 in range(2)
                ]
                for tw in range(2):
                    tcol = th * 1024 + tw * 512
                    for h2 in range(2):
                        off = h2 * 64
                        nc.tensor.matmul(
                            scps[h2][:, tw * 512 : (tw + 1) * 512],
                            lhsT=kt_sb[pp][off : off + 64, st * 128 : (st + 1) * 128],
                            rhs=qt_sb[pp][off : off + 64, tcol : tcol + 512],
                            start=True,
                            stop=True,
                        )
                ats = []
                for h2 in range(2):
                    at = at_pool.tile([128, 1024], bf16, tag="at", name="at")
                    nc.scalar.activation(
                        at[:], scps[h2][:], EXP,
                        bias=mb_sb[:, st : st + 1], scale=SCALE,
                    )
                    ats.append(at)
                return ats

            def emit_attnv(av_l, at, st, pp, h2):
                h = 2 * pp + h2
                for tw in range(2):
                    nc.tensor.matmul(
                        av_l[h2][tw][0 : DH + 1, :],
                        lhsT=v_sb[:, st, h, :],
                        rhs=at[:, tw * 512 : (tw + 1) * 512],
                        start=(st == 0),
                        stop=(st == nkt - 1),
                    )

            # normalize reads the pending phase's av psum directly
            def emit_normalize(pend):
                th_, pp_, av_ = pend
                sums = sums_pool.tile([97, 512], f32, tag="sums", name="sums")
                nc.gpsimd.memset(sums[:], 1.0)
                for h2 in range(2):
                    for tw in range(2):
                        r = 32 * (2 * tw + h2)
                        nc.vector.tensor_copy(
                            sums[r : r + 1, :], av_[h2][tw][DH : DH + 1, :]
                        )
                recips = sums_pool.tile([97, 512], f32, tag="recips", name="recips")
                nc.vector.reciprocal_approx_fast(recips[:], sums[:])
                for tw in range(2):
                    for h2 in range(2):
                        r = 32 * (2 * tw + h2)
                        tcol = th_ * 1024 + tw * 512
                        r_t = r_pool.tile([1, 512], f32, tag="r", name="r_t")
                        nc.vector.tensor_copy(r_t[0:1, :], recips[r : r + 1, :])
                        rb_t = rb_pool.tile([64, 512], f32, tag="rb", name="rb_t")
                        nc.gpsimd.partition_broadcast(rb_t[:], r_t[0:1, :])
                        nc.vector.tensor_mul(
                            outT_sb[h2 * 64 : (h2 + 1) * 64, pp_, tcol : tcol + 512],
                            av_[h2][tw][0:DH, :],
                            rb_t[:],
                        )

            def emit_wo(th_, tts=None):
                for tt in tts if tts is not None else range(th_ * 8, (th_ + 1) * 8):
                    ps = psA.tile([128, 512], f32, tag="mm", name="y_ps")
                    for c in range(HE // 128):
                        nc.tensor.matmul(
                            ps[:],
                            lhsT=outT_sb[:, c, tt * 128 : (tt + 1) * 128],
                            rhs=wo_sb[:, c, :],
                            start=(c == 0),
                            stop=(c == HE // 128 - 1),
                        )
                    y_sb = y_pool.tile([128, 512], f32, tag="y", name="y_sb")
                    nc.vector.tensor_copy(y_sb[:], ps[:])
                    nc.sync.dma_start(y[tt * 128 : (tt + 1) * 128, :], y_sb[:])

            # ---- prologue -----------------------------------------------
            emit_kt(0)
            emit_qt(0, 0)

            # ---- phases --------------------------------------------------
            phases = [(th, pp) for th in range(2) for pp in range(2)]
            pending = None

            # filler plan: (phase_i, st) -> list of emitters; av_st: when to
            # allocate the phase's av accumulators and drain the at backlog.
            filler_plan = {}
            v_jobs = [(lambda vst=vst: emit_v(vst)) for vst in range(nkt)]
            slots0 = max(nkt - 3, 1)
            per = (len(v_jobs) + slots0 - 1) // slots0
            for si in range(slots0):
                filler_plan[(0, min(si, nkt - 1))] = v_jobs[si * per : (si + 1) * per]
            filler_plan.setdefault((0, min(slots0, nkt - 1)), []).extend(
                [lambda: emit_kt(1), lambda: emit_qt(1, 0)]
            )
            av_st = {0: min(slots0, nkt - 1)}
            filler_plan[(1, min(2, nkt - 1))] = [lambda: emit_qt(0, 1)]
            av_st[1] = min(2, nkt - 1)
            filler_plan[(2, min(2, nkt - 1))] = [lambda: emit_qt(1, 1)]
            filler_plan[(2, min(3, nkt - 1))] = [lambda: emit_wo(0)]
            av_st[2] = min(3, nkt - 1)
            av_st[3] = min(1, nkt - 1)

            for phase_i, (th, pp) in enumerate(phases):
                av_l = None
                backlog = []

                def ensure_av():
                    nonlocal av_l
                    if av_l is not None:
                        return
                    av_l = [
                        [
                            psA.tile(
                                [128, 512], f32, tag="mm", name=f"av{h2}_{tw}"
                            )
                            for tw in range(2)
                        ]
                        for h2 in range(2)
                    ]
                    for at_, st_, h2_ in backlog:
                        emit_attnv(av_l, at_, st_, pp, h2_)
                    backlog.clear()

                for st in range(nkt):
                    ats = emit_scores(pp, th, st)
                    for h2, at in enumerate(ats):
                        if av_l is None:
                            backlog.append((at, st, h2))
                        else:
                            emit_attnv(av_l, at, st, pp, h2)
                    if st == 1 and pending is not None:
                        emit_normalize(pending)
                        pending = None
                    for fn in filler_plan.get((phase_i, st), []):
                        fn()
                    if st == av_st[phase_i]:
                        ensure_av()
                ensure_av()
                pending = (th, pp, av_l)

            # ---- tail: last phase normalize + Wo, pipelined by tw --------
            th_, pp_, av_ = pending
            sums = sums_pool.tile([97, 512], f32, tag="sums", name="sums")
            nc.gpsimd.memset(sums[:], 1.0)
            for h2 in range(2):
                for tw in range(2):
                    r = 32 * (2 * tw + h2)
                    nc.vector.tensor_copy(
                        sums[r : r + 1, :], av_[h2][tw][DH : DH + 1, :]
                    )
            recips = sums_pool.tile([97, 512], f32, tag="recips", name="recips")
            nc.vector.reciprocal_approx_fast(recips[:], sums[:])
            for tw in range(2):
                for h2 in range(2):
                    r = 32 * (2 * tw + h2)
                    tcol = th_ * 1024 + tw * 512
                    r_t = r_pool.tile([1, 512], f32, tag="r", name="r_t")
                    nc.vector.tensor_copy(r_t[0:1, :], recips[r : r + 1, :])
                    rb_t = rb_pool.tile([64, 512], f32, tag="rb", name="rb_t")
                    nc.gpsimd.partition_broadcast(rb_t[:], r_t[0:1, :])
                    nc.vector.tensor_mul(
                        outT_sb[h2 * 64 : (h2 + 1) * 64, pp_, tcol : tcol + 512],
                        av_[h2][tw][0:DH, :],
                        rb_t[:],
                    )
                emit_wo(th_, tts=range(th_ * 8 + tw * 4, th_ * 8 + (tw + 1) * 4))

    nc.compile()
    return nc


_NC_CACHE = {}
_LAST_STATE = {}


def _get_nc(nkt=None):
    if nkt is None:
        nkt = _LAST_STATE.get("nkt", 9)
    if nkt not in _NC_CACHE:
        _NC_CACHE[nkt] = build_nc(nkt)
    return _NC_CACHE[nkt]


def make_in_maps(x, mask, Wq, Wk, Wv, Wo):
    bf = ml_dtypes.bfloat16
    mask = np.asarray(mask)
    perms = []
    counts = []
    for b in range(B):
        valid = np.flatnonzero(mask[b] > 0)
        invalid = np.flatnonzero(mask[b] <= 0)
        perms.append(np.concatenate([valid, invalid]).astype(np.int64))
        counts.append(len(valid))
    nkt = max(1, int(np.ceil(max(counts) / 128)))
    nkt = min(nkt, S // 128)
    SV = nkt * 128
    _LAST_STATE["nkt"] = nkt
    _LAST_STATE["perms"] = perms

    # [H, D, DH] -> [D, H*DH]
    wq_f = np.ascontiguousarray(Wq.transpose(1, 0, 2).reshape(D, H * DH))
    wk_f = np.ascontiguousarray(Wk.transpose(1, 0, 2).reshape(D, H * DH))
    wv_f = np.ascontiguousarray(Wv.transpose(1, 0, 2).reshape(D, H * DH))
    in_maps = []
    for c in range(N_CORES):
        b, hg = c // 2, c % 2
        perm = perms[b]
        xT_p = np.ascontiguousarray(x[b].T[:, perm]).astype(bf)  # [D, S] permuted
        mb = np.where(mask[b][perm] > 0, 0.0, -MASK_NUM).astype(np.float32)[:SV]
        cols = slice(hg * HE, (hg + 1) * HE)
        in_maps.append(
            {
                "xT": xT_p,
                "wq": np.ascontiguousarray(wq_f[:, cols]).astype(bf),
                "wk": np.ascontiguousarray(wk_f[:, cols]).astype(bf),
                "wv": np.ascontiguousarray(wv_f[:, cols]).astype(bf),
                "wo": np.ascontiguousarray(Wo[cols, :]).astype(bf),
                "mbias": np.ascontiguousarray(mb),
            }
        )
    return in_maps


def combine_results(results):
    perms = _LAST_STATE["perms"]
    y = np.zeros((B, S, D), np.float32)
    for b in range(B):
        yp = results[2 * b]["y"] + results[2 * b + 1]["y"]
        y[b][perms[b]] = yp
    return y


def kernel(x, mask, Wq, Wk, Wv, Wo):
    in_maps = make_in_maps(
        np.asarray(x, np.float32),
        np.asarray(mask),
        np.asarray(Wq, np.float32),
        np.asarray(Wk, np.float32),
        np.asarray(Wv, np.float32),
        np.asarray(Wo, np.float32),
    )
    nc = _get_nc(_LAST_STATE["nkt"])
    res = run_bass_kernel_spmd(nc, in_maps, core_ids=list(range(N_CORES)))
    return combine_results(res.results)
